# revision 1
# baseline (speedup 1.0000x reference)
"""GAT-style node-feature kernel for Trainium2 (8 NeuronCores, SPMD).

Problem: y = Linear_o(MHA(x) with per-edge gate mask), where the mask is
mean(edge_gate, axis=-1) (B,V,V) applied multiplicatively to attention
scores before softmax.  edge_gate is (2,768,768,128) fp32 = 604 MB; its
HBM read dominates (memory-bound regime, ~211 us roofline at 358 GB/s
per core).

Sharding: the 1536 (batch*query) rows are split into 8 chunks of 192
(cores 0-3 -> batch 0, cores 4-7 -> batch 1).  Each core reads only its
75.5 MB slice of edge_gate, computes the channel-sum, and runs its
queries' attention against the full 768 keys of its batch (k/v
replicated per batch).  Host-side prep transposes the small weights/x
so the kernel needs no on-chip layout changes.

Key layout trick: scores are computed TRANSPOSED (keys on partitions,
queries on free dim).  The channel-reduce of edge_gate then lands
directly in the layout the mask-multiply needs (no transposes), softmax
denominators come from an appended ones-column in the value matrix, and
exp() needs no max-subtraction (|scores*mask| < ~10, fp32-safe).

Keys are processed in a permuted order (k' = r*128+p  <->  k = 6p+r, from
the contiguous-DMA-friendly reduce tiling); softmax and the attended sum
are permutation-invariant, so only the host-side column order of x^T
changes.

Optimization history (259 us baseline -> ~225 us measured):  the DMA
stream alone measures 213-222 us (at the HBM roofline), and every
For_i timing iteration ends in an all-engine barrier (no cross-
iteration overlap), so everything above the stream is reduce/attention
coupling plus the serial post-stream tail.  The shipped design:
  - "fold" reduce (KERNEL_RED=fold): GpSimd adds the two channel
    halves into a bf16 tile (SBUF only -- GpSimd cannot touch PSUM),
    then DVE reduce_sum reads 64 bf16 elements at 2x throughput; DVE
    per-group cost drops ~4x and stops being the straggler engine.
  - bf16 q/k/v/exp-weights and output projection (PE: 1 cycle/row vs
    fp32's 4); total rel-err ~4e-3 vs the 2e-2 gate.
  - attention in chunks of 64/64/32/32 queries; chunk 2 ends at query
    160 so its latency chain absorbs into the stream's engine gaps.
  - the final 32 queries' DMA groups taper (4,4,4,4,2x6,1x4) so the
    fold+reduce pipelines drain behind the small DMAs instead of
    piling up ~6 us of backlog after the last byte lands.
  - small chunks process head PAIRS (one PSUM tile, one fused exp per
    pair) and normalize heads 0-3 between pairs (their denominators
    are ready ~2 us before heads 4-7 finish); all latency-critical
    elementwise ops stay on DVE -- Tile's wait coalescing serializes
    alternating-engine chains anyway, at ~0.2 us per hop.
Constraints discovered the hard way: free-axis reduce_sum is DVE-only;
GPSIMD instructions cannot access PSUM (NCC_INLA001); PE transpose
outputs must start at PSUM partition 0 (NCC_IBIR151); engine copies
cannot shift partitions (lanes are locked).
"""

import numpy as np
from contextlib import ExitStack

P = 128          # partitions / hidden dim
HID = 128
NH = 8
HD = 16
V = 768          # keys per batch
NQ = 192         # queries per core
R = 6            # rows-per-partition in the reduce tiling (768 = 128*6)
# (start, size) of each DMA/reduce group.  The last groups are small so
# the final reduce (which sits on the critical tail) is short.
import os as _os
_GQ_CFG = _os.environ.get("KERNEL_GQ", "8tail")
if _GQ_CFG == "4":
    # small groups halve the fold/reduce pipelines' lag behind the DMA
    # stream; the tail taper drains them to ~1 query of trailing work
    GQ = 4
    GROUPS = [(i * 4, 4) for i in range(44)] + \
             [(176, 2), (178, 2), (180, 2), (182, 2), (184, 2), (186, 2),
              (188, 1), (189, 1), (190, 1), (191, 1)]
elif _GQ_CFG == "16":
    GQ = 16
    GROUPS = [(i * 16, 16) for i in range(8)] + \
             [(128, 16), (144, 16), (160, 16), (176, 8), (184, 8)]
elif _GQ_CFG == "12":
    GQ = 12
    GROUPS = [(i * 12, 12) for i in range(10)] + [(120, 8)] + \
             [(128 + i * 12, 12) for i in range(5)] + [(188, 4)]
elif _GQ_CFG == "8tail":
    # taper the final chunk's groups so the fold (Pool, 0.58x DMA rate)
    # and reduce (DVE) pipelines drain behind the small DMAs instead of
    # piling up backlog after the last byte lands
    GQ = 8
    GROUPS = [(i * 8, 8) for i in range(20)] + \
             [(160, 4), (164, 4), (168, 4), (172, 4),
              (176, 2), (178, 2), (180, 2), (182, 2), (184, 2), (186, 2),
              (188, 1), (189, 1), (190, 1), (191, 1)]
else:
    GQ = 8
    GROUPS = [(i * 8, 8) for i in range(24)]
EG_BUFS = int(_os.environ.get("KERNEL_EG_BUFS", "12" if GQ == 4 else "6"))
# precompute the last chunk's attention scores mid-stream and park them
# in PSUM.  Off by default: the PSUM banks it costs (psc 4->3, misc
# 2->1) slow the per-chunk head pipelines more than the parked scores
# save (cost-model timeline A/B).
PARK = _os.environ.get("KERNEL_PARK", "0") == "1"
# diagnostic modes: "full" (default), "dmaonly" (just the edge_gate
# stream), "dmared" (stream + reduces, no attention), "notail" (full
# minus the last chunk's attention -- isolates the serial tail)
KMODE = _os.environ.get("KERNEL_MODE", "full")
# attention chunks (start, size): the last ones are small because the
# final chunk's attention is the post-DMA tail
_CH_CFG = _os.environ.get("KERNEL_CHUNKS", "tail32")
if _CH_CFG == "tail16b":
    # 5 chunks: chunk 3's attention absorbs into the taper's DMA window,
    # leaving only 16 queries of attention behind the final reduce
    CHUNKS = [(0, 64), (64, 64), (128, 32), (160, 16), (176, 16)]
elif _CH_CFG == "tail32":
    CHUNKS = [(0, 64), (64, 64), (128, 32), (160, 32)]
elif _CH_CFG == "tail16":
    CHUNKS = [(0, 64), (64, 64), (128, 48), (176, 16)]
elif _CH_CFG == "tail8":
    CHUNKS = [(0, 64), (64, 64), (128, 56), (184, 8)]
else:
    CHUNKS = [(0, 64), (64, 64), (128, 64)]
# edge_gate channel-reduce strategy.  Free-axis reduce_sum is DVE-only
# and runs at 0.73x the DMA rate, making DVE the straggler engine.
# "fold": GpSimd (otherwise idle; tensor_tensor IS supported there) adds
# the two channel halves into a bf16 tile, then DVE reduces 64 bf16
# elements at 2x throughput -- DVE per-group cost drops ~4x.
RED_CFG = _os.environ.get("KERNEL_RED", "fold")
# bf16 for the q/k/v/wexp matmul path (PE: 1 cycle/row vs fp32's 4)
BF16 = _os.environ.get("KERNEL_BF16", "1") == "1"
N_CORES = 8
QSCALE = 1.0 / 512.0   # 1/sqrt(hd) * 1/channels = 1/4 * 1/128

_cached = {}


def _build_module(repeat=1):
    """Build the per-core Bass module.

    repeat > 1 wraps the whole body in a hardware For_i loop re-running it
    on identical inputs -- used only for timing (amortizes host dispatch).
    """
    import concourse.bass as bass
    import concourse.tile as tile
    from concourse import bacc, mybir
    from concourse.masks import make_identity
    from contextlib import nullcontext

    f32 = mybir.dt.float32
    bf16 = mybir.dt.bfloat16
    adt = bf16 if BF16 else f32      # attention-path dtype
    AFT = mybir.ActivationFunctionType
    AX = mybir.AxisListType

    nc = bacc.Bacc("TRN2", target_bir_lowering=False, debug=False)

    eg = nc.dram_tensor("eg", [NQ, V, HID], f32, kind="ExternalInput").ap()
    xqT = nc.dram_tensor("xqT", [P, NQ], f32, kind="ExternalInput").ap()
    xkT = nc.dram_tensor("xkT", [P, V], f32, kind="ExternalInput").ap()
    wqT = nc.dram_tensor("wqT", [P, P], f32, kind="ExternalInput").ap()
    wkT = nc.dram_tensor("wkT", [P, P], f32, kind="ExternalInput").ap()
    wvT = nc.dram_tensor("wvT", [P, P], f32, kind="ExternalInput").ap()
    woT = nc.dram_tensor("woT", [P, P], f32, kind="ExternalInput").ap()
    bqs = nc.dram_tensor("bqs", [HD, NH], f32, kind="ExternalInput").ap()
    bkc = nc.dram_tensor("bkc", [HD, NH], f32, kind="ExternalInput").ap()
    bvr = nc.dram_tensor("bvr", [1, P], f32, kind="ExternalInput").ap()
    bor = nc.dram_tensor("bor", [1, P], f32, kind="ExternalInput").ap()
    out = nc.dram_tensor("out", [NQ, HID], f32, kind="ExternalOutput").ap()

    # edge_gate viewed so partition p holds rows 6p..6p+5 of each query's
    # (768,128) block: 3 KB contiguous per partition per query.
    eg_r = eg.rearrange("q (p r) c -> p q r c", p=P)

    with tile.TileContext(nc) as tc, ExitStack() as ctx:
        singles = ctx.enter_context(tc.tile_pool(name="singles", bufs=1))
        egp = ctx.enter_context(tc.tile_pool(name="egp", bufs=EG_BUFS))
        # >= tiles-per-chunk slots: recipA/recipB/att_sb/attT_sb/y_sb
        # all rotate here; a shallow ring makes tile-slot reuse serialize
        # the normalize across engines (measured on the cost-model timeline)
        workp = ctx.enter_context(tc.tile_pool(
            name="work", bufs=int(_os.environ.get("KERNEL_WORK_BUFS", "13"))))
        wexpp = ctx.enter_context(tc.tile_pool(
            name="wexp", bufs=int(_os.environ.get("KERNEL_WEXP_BUFS", "6"))))
        use_park = PARK and KMODE == "full"
        # PSUM banks: sc*3 + park*2 + acc*2 + misc*1 = 8 when parking
        pp_sc = ctx.enter_context(tc.tile_pool(
            name="psc",
            bufs=3 if use_park else int(_os.environ.get("KERNEL_PSC_BUFS", "4")),
            space="PSUM"))
        pp_acc = ctx.enter_context(tc.tile_pool(name="pacc", bufs=2, space="PSUM"))
        pp_misc = ctx.enter_context(tc.tile_pool(
            name="pmisc", bufs=1 if use_park else 2, space="PSUM"))
        pp_park = (ctx.enter_context(tc.tile_pool(name="ppark", bufs=1,
                                                  space="PSUM"))
                   if use_park else None)

        if repeat == 1:
            loop_cm = nullcontext()
        else:
            ET = mybir.EngineType
            loop_cm = tc.For_i(0, repeat, 1,
                               hint_engines=(ET.PE, ET.DVE, ET.Activation,
                                             ET.SP, ET.Pool))
        ctx.enter_context(loop_cm)

        # ---- constants / small inputs ----
        wqT_t = singles.tile([P, P], f32)
        nc.scalar.dma_start(wqT_t[:], wqT)
        wkT_t = singles.tile([P, P], f32)
        nc.scalar.dma_start(wkT_t[:], wkT)
        wvT_t = singles.tile([P, P], f32)
        nc.scalar.dma_start(wvT_t[:], wvT)
        woT_t = singles.tile([P, P], f32)
        nc.scalar.dma_start(woT_t[:], woT)
        xqT_t = singles.tile([P, NQ], f32)
        nc.scalar.dma_start(xqT_t[:], xqT)
        xkT_t = singles.tile([P, V], f32)
        nc.scalar.dma_start(xkT_t[:], xkT)
        bqs_t = singles.tile([HD, NH], f32)
        nc.scalar.dma_start(bqs_t[:], bqs)
        bkc_t = singles.tile([HD, NH], f32)
        nc.scalar.dma_start(bkc_t[:], bkc)
        bvr_t = singles.tile([1, P], f32)
        nc.scalar.dma_start(bvr_t[:], bvr)
        bor_t = singles.tile([1, P], f32)
        nc.scalar.dma_start(bor_t[:], bor)

        ones_t = singles.tile([1, P], f32)
        nc.vector.memset(ones_t[:], 1.0)
        ident = singles.tile([P, P], f32)
        make_identity(nc, ident[:])
        # bf16 copies for the output projection (PE: 1 cycle/row vs 4)
        ones_bf = singles.tile([1, P], adt)
        nc.vector.memset(ones_bf[:], 1.0)
        woT_bf = singles.tile([P, P], adt)
        nc.scalar.copy(woT_bf[:], woT_t[:])
        bor_bf = singles.tile([1, P], adt)
        nc.scalar.copy(bor_bf[:], bor_t[:])

        # channel-sums of edge_gate: mbuf[p, t, r] = sum_c eg[t, 6p+r, c],
        # one tile per attention chunk
        mbufs = [singles.tile([P, csz, R], f32, name=f"mbuf{i}", tag=f"mbuf{i}")
                 for i, (_, csz) in enumerate(CHUNKS)]

        # head-major layouts (PE operands must start at partition 0)
        qT_t = singles.tile([HD, NH, NQ], adt)   # (d, head, query), scaled 1/512
        kT_t = singles.tile([HD, NH, V], adt)    # (d, head, key') permuted keys
        # v matrix with a ones-column appended per head: (k', head, 17)
        v_aug = singles.tile([P, R, NH, HD + 1], adt)

        # ---- q/k/v projections ----
        # per-head matmuls: engine reads must start at 32-aligned partitions,
        # so (16, ...) operands live at partition base 0 and heads are
        # separated via free-dim slices of the transposed weights.
        for h in range(NH):
            qps = pp_misc.tile([HD, NQ], f32, tag="misc")
            nc.tensor.matmul(qps[:], wqT_t[:, h * HD:(h + 1) * HD], xqT_t[:],
                             start=True, stop=True)
            nc.scalar.activation(qT_t[:, h, :], qps[:], AFT.Identity,
                                 bias=bqs_t[:, h:h + 1], scale=QSCALE)
            for half in range(2):
                kps = pp_misc.tile([HD, 384], f32, tag="misc")
                nc.tensor.matmul(kps[:], wkT_t[:, h * HD:(h + 1) * HD],
                                 xkT_t[:, 384 * half:384 * (half + 1)],
                                 start=True, stop=True)
                nc.scalar.activation(kT_t[:, h, 384 * half:384 * (half + 1)],
                                     kps[:], AFT.Identity,
                                     bias=bkc_t[:, h:h + 1], scale=1.0)

        nc.vector.memset(v_aug[:], 1.0)   # ones-columns survive the copies below
        for j in range(R):
            vps = pp_misc.tile([P, P], f32, tag="misc")
            nc.tensor.matmul(vps[:], ones_t[:], bvr_t[:], start=True, stop=False)
            nc.tensor.matmul(vps[:], xkT_t[:, j * P:(j + 1) * P], wvT_t[:],
                             start=False, stop=True)
            nc.scalar.copy(v_aug[:, j, :, 0:HD],
                           vps[:].rearrange("p (h d) -> p h d", h=NH))

        # ---- attention for one chunk of queries ----
        # Emitted with a large priority offset (when enabled) so the Tile
        # scheduler prefers the DMA+reduce stream whenever both are ready;
        # attention then fills engine gaps instead of stalling the stream.
        atn_lowpri = _os.environ.get("KERNEL_ATN_LOWPRI", "1") == "1"

        # scores for the last chunk, precomputed mid-stream (no edge_gate
        # dependency) so the post-stream tail skips the PE score matmuls
        # emitted at stream priority: PE is idle during the stream, and these
        # must land well before the tail (low priority deferred them INTO the
        # tail, gating the last chunk -- measured on the cost-model timeline)
        park = None
        if use_park:
            t0, tcq = CHUNKS[-1]
            park = pp_park.tile([P, NH, R, tcq], f32)
            for h in range(NH):
                for j in range(R):
                    nc.tensor.matmul(
                        park[:, h, j, :],
                        kT_t[:, h, j * P:(j + 1) * P],
                        qT_t[:, h, t0:t0 + tcq],
                        start=True, stop=True)

        def attention(ci):
            saved_pri = tc.cur_priority
            if atn_lowpri:
                tc.cur_priority = 1_000_000 + ci * 10_000
            t0, tcq = CHUNKS[ci]
            parked = park is not None and ci == len(CHUNKS) - 1
            mb = mbufs[ci]
            aug = pp_acc.tile([tcq, NH, HD + 1], f32)
            # mask viewed as (p, block, query) to match the scores layout
            mbT = mb[:, :, :].rearrange("p t r -> p r t")
            # small chunks (the taper tail): two heads share one PSUM tile,
            # the exp is one fused op per pair, and the mask-muls split
            # across DVE (heads 0-3) and GpSimd (heads 4-7) so the tail's
            # serial mul block halves.  Large chunks keep the per-head form
            # (a 2-head 64q tile would exceed a 2KB PSUM bank).
            pair = (not parked) and tcq <= 32
            if pair:
                def emit_pair(hh):
                    h0b = 2 * hh
                    sc2 = pp_sc.tile([P, 2, R, tcq], f32, tag="sc")
                    for i in range(2):
                        for j in range(R):
                            nc.tensor.matmul(
                                sc2[:, i, j, :],
                                kT_t[:, h0b + i, j * P:(j + 1) * P],
                                qT_t[:, h0b + i, t0:t0 + tcq],
                                start=True, stop=True)
                    # GpSimd cannot access PSUM (BIR verifier), so all
                    # mask-muls run on DVE
                    nc.vector.tensor_mul(sc2[:, 0], sc2[:, 0], mbT)
                    nc.vector.tensor_mul(sc2[:, 1], sc2[:, 1], mbT)
                    wexp2 = wexpp.tile([P, 2, R, tcq], adt, tag="wexp")
                    nc.scalar.activation(wexp2[:], sc2[:], AFT.Exp)
                    for i in range(2):
                        for j in range(R):
                            nc.tensor.matmul(aug[:, h0b + i, :],
                                             wexp2[:, i, j, :],
                                             v_aug[:, j, h0b + i, :],
                                             start=(j == 0), stop=(j == R - 1))

                # heads 0-3 finish first and their normalize (all DVE --
                # cross-engine sems serialize anyway) overlaps heads 4-7
                emit_pair(0)
                emit_pair(1)
                recipA = workp.tile([tcq, NH // 2], f32, tag="recA")
                nc.vector.reciprocal(recipA[:], aug[:, 0:NH // 2, HD])
                # one tile, one transpose: PE transpose outputs must start at
                # PSUM partition 0 (NCC_IBIR151), so a per-half transpose
                # into partitions 64-127 is illegal, and engine copies are
                # lane-locked (cannot shift partitions)
                att_sb = workp.tile([tcq, P], f32, tag="att_sb")
                for h in range(NH // 2):
                    nc.vector.tensor_scalar_mul(
                        att_sb[:, h * HD:(h + 1) * HD],
                        aug[:, h, 0:HD], recipA[:, h:h + 1])
                emit_pair(2)
                emit_pair(3)
                # all normalize work stays on DVE: cross-engine alternatives
                # get serialized anyway by Tile's wait coalescing and cost
                # an extra ~0.2us per hop (cost-model A/B)
                recipB = workp.tile([tcq, NH // 2], f32, tag="recB")
                nc.vector.reciprocal(recipB[:], aug[:, NH // 2:NH, HD])
                for h in range(NH // 2, NH):
                    nc.vector.tensor_scalar_mul(
                        att_sb[:, h * HD:(h + 1) * HD],
                        aug[:, h, 0:HD], recipB[:, h - NH // 2:h - NH // 2 + 1])
                attT_ps = pp_misc.tile([P, 128], f32, tag="misc")
                attT_sb = workp.tile([P, 128], adt)
                nc.tensor.transpose(attT_ps[:, 0:tcq], att_sb[:],
                                    ident[0:tcq, 0:tcq])
                nc.scalar.copy(attT_sb[:, 0:tcq], attT_ps[:, 0:tcq])
                yps = pp_misc.tile([128, P], f32, tag="misc")
                nc.tensor.matmul(yps[0:tcq, :], ones_bf[0:1, 0:tcq],
                                 bor_bf[:], start=True, stop=False)
                nc.tensor.matmul(yps[0:tcq, :], attT_sb[:, 0:tcq], woT_bf[:],
                                 start=False, stop=True)
                y_sb = workp.tile([128, P], f32)
                nc.vector.tensor_copy(y_sb[0:tcq, :], yps[0:tcq, :])
                nc.scalar.dma_start(out[t0:t0 + tcq, :], y_sb[0:tcq, :])
                if atn_lowpri:
                    tc.cur_priority = saved_pri
                return
            else:
                for h in range(NH):
                    # one PSUM tile holds all 6 key-blocks' transposed scores
                    # so the mask-multiply and exp are single fat ops (the
                    # per-block version was latency-bound on hops)
                    if parked:
                        scv = park[:, h, :, :]
                    else:
                        sc = pp_sc.tile([P, R, tcq], f32, tag="sc")
                        for j in range(R):
                            nc.tensor.matmul(
                                sc[:, j, :],
                                kT_t[:, h, j * P:(j + 1) * P],
                                qT_t[:, h, t0:t0 + tcq],
                                start=True, stop=True)
                        scv = sc[:]
                    nc.vector.tensor_mul(scv, scv, mbT)
                    wexp = wexpp.tile([P, R, tcq], adt, tag="wexp")
                    nc.scalar.activation(wexp[:], scv, AFT.Exp)
                    for j in range(R):
                        nc.tensor.matmul(aug[:, h, :], wexp[:, j, :],
                                         v_aug[:, j, h, :],
                                         start=(j == 0), stop=(j == R - 1))
            # heads 0-3 normalize on DVE, heads 4-7 on ACT, into separate
            # tiles: same-engine ordering is free, and no cross-engine sem
            # chain is induced by Tile's wait coalescing (alternating the
            # engines per head serialized all eight ops pairwise).  recipA
            # (heads 0-3) is emitted first: its denominators land ~2us
            # before heads 4-7 finish.
            # recipA (heads 0-3) first: its denominators land ~2us before
            # heads 4-7 finish.  Normalize all on DVE into one tile; the
            # single transpose must output at PSUM partition 0 (NCC_IBIR151).
            recipA = workp.tile([tcq, NH // 2], f32, tag="recA")
            nc.vector.reciprocal(recipA[:], aug[:, 0:NH // 2, HD])
            att_sb = workp.tile([tcq, P], f32, tag="att_sb")
            for h in range(NH // 2):
                nc.vector.tensor_scalar_mul(
                    att_sb[:, h * HD:(h + 1) * HD],
                    aug[:, h, 0:HD], recipA[:, h:h + 1])
            recipB = workp.tile([tcq, NH // 2], f32, tag="recB")
            nc.vector.reciprocal(recipB[:], aug[:, NH // 2:NH, HD])
            for h in range(NH // 2, NH):
                nc.vector.tensor_scalar_mul(
                    att_sb[:, h * HD:(h + 1) * HD],
                    aug[:, h, 0:HD], recipB[:, h - NH // 2:h - NH // 2 + 1])
            attT_ps = pp_misc.tile([P, 128], f32, tag="misc")
            attT_sb = workp.tile([P, 128], adt)
            nc.tensor.transpose(attT_ps[:, 0:tcq], att_sb[:],
                                ident[0:tcq, 0:tcq])
            nc.scalar.copy(attT_sb[:, 0:tcq], attT_ps[:, 0:tcq])
            yps = pp_misc.tile([128, P], f32, tag="misc")
            nc.tensor.matmul(yps[0:tcq, :], ones_bf[0:1, 0:tcq], bor_bf[:],
                             start=True, stop=False)
            nc.tensor.matmul(yps[0:tcq, :], attT_sb[:, 0:tcq], woT_bf[:],
                             start=False, stop=True)
            y_sb = workp.tile([128, P], f32)
            nc.vector.tensor_copy(y_sb[0:tcq, :], yps[0:tcq, :])
            nc.scalar.dma_start(out[t0:t0 + tcq, :], y_sb[0:tcq, :])
            if atn_lowpri:
                tc.cur_priority = saved_pri

        foldp = (ctx.enter_context(tc.tile_pool(name="fold", bufs=2))
                 if RED_CFG == "fold" else None)

        def reduce_group(mb_out, egt, gq):
            if RED_CFG == "fold":
                tmp = foldp.tile([P, GQ, R, HID // 2], bf16, tag="fold")
                nc.gpsimd.tensor_add(tmp[:, 0:gq, :, :],
                                     egt[:, 0:gq, :, 0:HID // 2],
                                     egt[:, 0:gq, :, HID // 2:HID])
                nc.vector.reduce_sum(mb_out, tmp[:, 0:gq, :, :], axis=AX.X)
            else:
                nc.vector.reduce_sum(mb_out, egt[:, 0:gq, :, :], axis=AX.X)

        # ---- main stream: DMA edge_gate slices + channel-sum reduce ----
        alt_rings = _os.environ.get("KERNEL_ALT_RINGS", "0") == "1"
        for gi, (q0, gq) in enumerate(GROUPS):
            egt = egp.tile([P, GQ, R, HID], f32, tag="eg")
            eng = nc.scalar if (alt_rings and gi % 2) else nc.sync
            eng.dma_start(egt[:, 0:gq, :, :], eg_r[:, q0:q0 + gq, :, :])
            if KMODE == "dmaonly":
                continue
            ci = max(i for i, (c0, _) in enumerate(CHUNKS) if c0 <= q0)
            c0 = CHUNKS[ci][0]
            reduce_group(mbufs[ci][:, q0 - c0:q0 - c0 + gq, :], egt, gq)
            if KMODE in ("full", "notail") and any(q0 + gq == c0 + csz
                                                   for (c0, csz) in CHUNKS):
                done = next(i for i, (c0, csz) in enumerate(CHUNKS)
                            if q0 + gq == c0 + csz)
                if KMODE == "full" or done < len(CHUNKS) - 1:
                    attention(done)
        if KMODE != "full":
            ytmp = workp.tile([P, HID], f32)
            nc.vector.memset(ytmp[:], 0.0)
            nc.scalar.dma_start(out[0:128, :], ytmp[:])
            nc.scalar.dma_start(out[128:192, :], ytmp[0:64, :])

    nc.compile()
    return nc


def _get_module(repeat=1):
    if repeat not in _cached:
        _cached[repeat] = _build_module(repeat)
    return _cached[repeat]


def _prep_in_maps(x, edge_gate, Wq, bq, Wk, bk, Wv, bv, Wo, bo):
    x = np.asarray(x, dtype=np.float32)
    edge_gate = np.asarray(edge_gate, dtype=np.float32)

    # permuted key order: column j of xkT is original key 6*(j%128) + j//128
    jj = np.arange(V)
    perm = 6 * (jj % P) + jj // P

    common = {
        "wqT": np.ascontiguousarray(np.asarray(Wq, np.float32).T),
        "wkT": np.ascontiguousarray(np.asarray(Wk, np.float32).T),
        "wvT": np.ascontiguousarray(np.asarray(Wv, np.float32).T),
        "woT": np.ascontiguousarray(np.asarray(Wo, np.float32).T),
        "bqs": np.ascontiguousarray((np.asarray(bq, np.float32) * QSCALE)
                                    .reshape(NH, HD).T),
        "bkc": np.ascontiguousarray(np.asarray(bk, np.float32).reshape(NH, HD).T),
        "bvr": np.ascontiguousarray(np.asarray(bv, np.float32).reshape(1, P)),
        "bor": np.ascontiguousarray(np.asarray(bo, np.float32).reshape(1, P)),
    }

    in_maps = []
    for c in range(N_CORES):
        b = c // 4
        q0 = (c % 4) * NQ
        xb = x[b]
        m = dict(common)
        m["eg"] = np.ascontiguousarray(edge_gate[b, q0:q0 + NQ])
        m["xqT"] = np.ascontiguousarray(xb[q0:q0 + NQ].T)
        m["xkT"] = np.ascontiguousarray(xb[perm].T)
        in_maps.append(m)
    return in_maps


def kernel(x, edge_gate, Wq, bq, Wk, bk, Wv, bv, Wo, bo):
    from concourse.bass_utils import run_bass_kernel_spmd

    x = np.asarray(x, dtype=np.float32)
    B, Vv, H = x.shape
    in_maps = _prep_in_maps(x, edge_gate, Wq, bq, Wk, bk, Wv, bv, Wo, bo)

    nc = _get_module()
    res = run_bass_kernel_spmd(nc, in_maps, core_ids=list(range(N_CORES)))
    y = np.stack([r["out"] for r in res.results], axis=0)  # (8, 192, 128)
    return y.reshape(B, Vv, H)

